# revision 20
# baseline (speedup 1.0000x reference)
"""BitSelfAttention on 8 TRN2 NeuronCores.

Sharding: core c handles batch b = c//2 and head-group hg = c%2 (8 of 16 heads).
Each core computes its 8 heads' QKV projections + causal attention + its slice
of the o_proj GEMM, producing a partial output (transposed, [D, T], fp32).
Host sums the two head-group partials per batch at the end.

Speed: the QKV projection GEMMs run a mixed-precision contraction: the first
N1=6 of 16 k-tiles of x are fp8(e4m3) and processed pairwise with
perf_mode=DoubleRow (2 fp8 MACs/cell/cycle, ~1.7x bf16 for those tiles); the
remaining 10 k-tiles stay bf16. BitLinear weights are pure ternary {-1,0,+1}
— exact in both fp8 and bf16 — so the only quantization error is on the fp8
slice of x (measured end-to-end rel-err ~1.5e-2 vs the 2e-2 gate).
gamma scales are applied exactly in fp32: gamma_q*gamma_k folds into the
softmax exp scale, gamma_v into the row-sum normalization, gamma_o stays
folded into the bf16 o_proj weights.

Device layouts (per core):
  x8   [128, N1, T] fp8 : x[b].T k-tiles 0..N1 (DoubleRow pairs on dim 1)
  xT   [NB*128, T] bf16 : x[b].T k-tiles N1..16
  w*8  [H, 128, N1*128] fp8  : ternary weight k-tiles 0..N1 (per head)
  w*T  [H, 128, NB*128] bf16 : ternary weight k-tiles N1..16
  woT  [MT, 128, H*128] bf16 : w_o_eff[:, hg-cols].T (gamma_o folded)
  cmask[4, 128, 512] bf16 : causal masks for the 4 diagonal offsets
  outT [D, T]  fp32 : partial output, transposed

Per head h: Q^T,K^T [dh=128, T] (dh-major), V^T transposed on the PE into
token-major V tiles. Attention computed as S^T = K^T_tile.T @ Q^T_block so
softmax rows land on the free axis; P^T = exp(S^T*scale) (ACT, PSUM->SBUF
bf16); key-tile partial row-sums accumulate in fp32 on the vector engine and
one all-ones stationary matmul per block reduces across partitions while
broadcasting the result to every partition; O^T = V_tile.T @ P^T accumulated
over key tiles; normalize with fast-reciprocal+multiply during PSUM eviction.
o_proj consumes O^T tiles directly as stationary operands, producing outT; its
per-token-block chains double as PE fill work zipped into the last head's
attention, just as each head's projection chains are zipped into the previous
head's attention (the attention inner loop is otherwise exp-latency-gated on
the in-order PE).
"""

import math

import ml_dtypes
import numpy as np

import concourse.mybir as mybir
import concourse.tile as tile
from concourse import bacc
from concourse import bass_utils
from concourse.masks import make_identity

BF16 = mybir.dt.bfloat16
FP8 = mybir.dt.float8e4
F32 = mybir.dt.float32

D_MODEL = 2048
N_HEAD = 16
D_HEAD = 128
B = 4
T_FULL = 2048
N_CORES = 8
F_LOC = D_MODEL // 2  # features per core (8 heads)
N1 = 6                # fp8 k-tiles (of 16) in the QKV projection contraction


def build_bass(s_exp, inv_gv, T=T_FULL, D=D_MODEL, F=F_LOC, debug=False):
    """Build the single-core program (SPMD across 8 cores via input data).

    s_exp: softmax exp scale = gamma_q*gamma_k/sqrt(dh) (fp32 immediate)
    inv_gv: 1/gamma_v, folded into the row-sum (fp32 immediate)
    """
    P = 128
    KD = D // P      # contraction 128-tiles
    NB = KD - N1     # bf16 k-tiles
    TT = T // P      # token 128-tiles
    TB = T // 512    # token 512-blocks
    H = F // P       # local heads
    MT = D // P      # output-dmodel 128-tiles
    KT_PER_B = 512 // P
    DR = mybir.MatmulPerfMode.DoubleRow

    nc = bacc.Bacc("TRN2", target_bir_lowering=False, debug=debug,
                   num_devices=N_CORES)
    x8_d = nc.dram_tensor("x8", [P, N1, T], FP8, kind="ExternalInput").ap()
    xb_d = nc.dram_tensor("xb", [P, NB, T], BF16, kind="ExternalInput").ap()
    # weights pre-tiled on host into the exact SBUF layouts. QKV packed per
    # head into one fp8 + one bf16 tensor (2 DMA issues per head instead of
    # 6 — each dma_start costs ~0.6us of Sync-engine issue time):
    #   w8: [H, 128, pi, j*128+f] = w_tern[pi][h*128+f, j*128+p]   (V,Q,K)
    #   wb: [H, 128, pi, kd*128+f] = w_tern[pi][h*128+f, (N1+kd)*128+p]
    #   woT: [MT, 128, H*128] with [m, p, h*128+j] = wo_eff[m*128+j, h*128+p]
    w8_d = nc.dram_tensor("w8", [H, P, 3 * N1 * P], FP8,
                          kind="ExternalInput").ap()
    wb_d = nc.dram_tensor("wb", [H, P, 3 * NB * P], BF16,
                          kind="ExternalInput").ap()
    woT_d = nc.dram_tensor("woT", [MT, P, H * P], BF16,
                           kind="ExternalInput").ap()
    cm_d = nc.dram_tensor("cmask", [4, P, 512], BF16, kind="ExternalInput").ap()
    # bf16 partials: host sums the two head-group halves in fp32; the bf16
    # rounding of each partial is ~2e-4 of the output absmax — negligible —
    # and it halves the 16MB output DMA stream.
    out_d = nc.dram_tensor("outT", [D, T], BF16, kind="ExternalOutput").ap()

    with tile.TileContext(nc) as tc:
        with (
            tc.tile_pool(name="big", bufs=1) as big,
            tc.tile_pool(name="work", bufs=2) as work,
            tc.tile_pool(name="psS", bufs=3, space="PSUM") as psS,
            tc.tile_pool(name="psO", bufs=2, space="PSUM") as psO,
            tc.tile_pool(name="psR", bufs=1, space="PSUM") as psR,
            tc.tile_pool(name="psP", bufs=2, space="PSUM") as psP,
        ):
            # ---- head-0 weights first: the first matmul chains need them
            def load_head_weights(h):
                # bufs=4: heads h..h+3 alive (prefetch depth 3)
                w8h = work.tile([P, 3, N1, P], FP8, name=f"w8_{h}", tag="w8",
                                bufs=4)
                nc.sync.dma_start(out=w8h.rearrange("p i n f -> p (i n f)"),
                                  in_=w8_d[h])
                wbh = work.tile([P, 3, NB, P], BF16, name=f"wb_{h}", tag="wb",
                                bufs=4)
                nc.sync.dma_start(out=wbh.rearrange("p i n f -> p (i n f)"),
                                  in_=wb_d[h])
                return (w8h, wbh)

            ws0 = load_head_weights(0)
            ones = big.tile([P, P], BF16, name="ones_sb", tag="ones", bufs=1)
            nc.vector.memset(ones, 1.0)
            ident = big.tile([P, P], BF16, name="ident_sb", tag="ident", bufs=1)
            make_identity(nc, ident)
            # ---- x: chunked by 512-token block so the first projection
            # chains start after ~1/4 of the x DMA instead of all of it.
            # Block 0 is issued first; head-1 weights go next (ahead of the
            # remaining x blocks) so the early chains never wait on a
            # weight DMA queued behind 5MB of x.
            x8 = big.tile([P, N1, T], FP8, name="x8_sb", tag="x8", bufs=1)
            xb = big.tile([P, NB, T], BF16, name="xb_sb", tag="xb", bufs=1)

            def dma_x_block(tb):
                ts_ = slice(tb * 512, (tb + 1) * 512)
                nc.sync.dma_start(out=x8[:, :, ts_], in_=x8_d[:, :, ts_])
                nc.sync.dma_start(out=xb[:, :, ts_], in_=xb_d[:, :, ts_])

            dma_x_block(0)
            cmask = big.tile([P, 4, 512], BF16, name="cmask_sb", tag="cmask",
                             bufs=1)
            for i in range(4):
                nc.sync.dma_start(out=cmask[:, i, :], in_=cm_d[i])
            ot = [big.tile([P, T], BF16, name=f"ot{h}", tag="ot", bufs=H)
                  for h in range(H)]

            # ---- per-head pipeline with cross-head fill interleaving.
            # The attention inner loop is ACT(exp)-gated by ~40ns/iter; we
            # pump one projection matmul of the NEXT head between attention
            # iterations so the (in-order) PE always has fill work.
            def alloc_head_tiles(h):
                vT = work.tile([P, T], BF16, name=f"vT{h}", tag="vT")
                vh = work.tile([P, TT, P], BF16, name=f"vh{h}", tag="vh")
                qt_ = work.tile([P, T], BF16, name=f"qt{h}", tag="qt")
                kt_ = work.tile([P, T], BF16, name=f"kt{h}", tag="kt")
                return vT, vh, qt_, kt_

            def proj_fill_gen(ws, tiles):
                """V^T/Q^T/K^T projection chains in token-block-outer order
                (so head 0 tracks the chunked x DMA arrival), yielding after
                every matmul so the caller can interleave them. Each chain:
                NB bf16 matmuls + N1/2 fp8 DoubleRow matmuls into one PSUM
                accumulation group."""
                w8h, wbh = ws
                vT, vh, qt_, kt_ = tiles
                for tb in range(TB):
                    for pi, dst in ((0, vT), (1, qt_), (2, kt_)):
                        ts_ = slice(tb * 512, (tb + 1) * 512)
                        ps = psP.tile([P, 512], F32, name="psfill", tag="psp")
                        for kd in range(NB):
                            nc.tensor.matmul(ps, lhsT=wbh[:, pi, kd, :],
                                             rhs=xb[:, kd, ts_],
                                             start=(kd == 0), stop=False)
                            yield
                        for j in range(N1 // 2):
                            nc.tensor.matmul(
                                ps, lhsT=w8h[:, pi, 2 * j:2 * j + 2, :],
                                rhs=x8[:, 2 * j:2 * j + 2, ts_],
                                start=False, stop=(j == N1 // 2 - 1),
                                perf_mode=DR)
                            yield
                        nc.vector.tensor_copy(out=dst[:, ts_], in_=ps)

            def pump(gen, n):
                for _ in range(n):
                    try:
                        next(gen)
                    except StopIteration:
                        return False
                return True

            def pump_n(gen, n):
                c = 0
                for _ in range(n):
                    try:
                        next(gen)
                        c += 1
                    except StopIteration:
                        break
                return c

            def oproj_nb_gen(nb):
                """o_proj chains for one token block (needs all heads' ot
                columns of that block only), yielding per matmul. woh weight
                tiles are prefetched two m-tiles ahead so the chains never
                stall the PE on their DMA."""
                ns = slice(nb * 512, (nb + 1) * 512)
                wohs = {}

                def load(m):
                    woh = work.tile([P, H, P], BF16, name=f"woh{nb}_{m}",
                                    tag="woh", bufs=4)
                    nc.sync.dma_start(out=woh.rearrange("p h f -> p (h f)"),
                                      in_=woT_d[m])
                    return woh

                wohs[0] = load(0)
                wohs[1] = load(1)
                yield  # let attention matmuls cover the woh DMA latency
                for m in range(MT):
                    if m + 2 < MT:
                        wohs[m + 2] = load(m + 2)
                    ps = psP.tile([P, 512], F32, name="psout", tag="psp")
                    for hh in range(H):
                        nc.tensor.matmul(ps, lhsT=wohs[m][:, hh, :],
                                         rhs=ot[hh][:, ns],
                                         start=(hh == 0), stop=(hh == H - 1))
                        yield
                    del wohs[m]
                    stg = work.tile([P, 512], BF16, name="ostage",
                                    tag="ostage", bufs=4)
                    nc.vector.tensor_copy(out=stg, in_=ps)
                    nc.sync.dma_start(out=out_d[m * P:(m + 1) * P, ns],
                                      in_=stg)

            # head-1 weights load right after x block 0; the remaining x
            # blocks stream behind them
            ws_list = [None] * (H + 3)
            ws_list[0] = ws0
            if H > 1:
                ws_list[1] = load_head_weights(1)
            for tb in range(1, TB):
                dma_x_block(tb)
            cur_tiles = alloc_head_tiles(0)
            g0 = proj_fill_gen(ws_list[0], cur_tiles)
            while pump(g0, 1):
                pass

            fills = []

            def pump_fills(n):
                while n > 0 and fills:
                    n -= pump_n(fills[0], n)
                    if n > 0:
                        fills.pop(0)

            for h in range(H):
                vT, vh, qt_, kt_ = cur_tiles
                # prefetch weights three heads ahead so fill matmuls never
                # wait on their DMA (a blocked fill stalls the in-order PE)
                if h == 0 and H > 2:
                    ws_list[2] = load_head_weights(2)
                if h + 3 < H:
                    ws_list[h + 3] = load_head_weights(h + 3)
                if h + 1 < H:
                    next_tiles = alloc_head_tiles(h + 1)
                    fills.append(proj_fill_gen(ws_list[h + 1], next_tiles))
                else:
                    next_tiles = None

                def emit_transpose(kt):
                    # lives in the psS pool: psP slots are held long by
                    # in-flight interleaved fill chains
                    pst = psS.tile([P, 512], BF16, name="pst", tag="pss")
                    nc.tensor.transpose(pst[:, 0:P],
                                        vT[:, kt * P:(kt + 1) * P], ident)
                    nc.vector.tensor_copy(out=vh[:, kt, :], in_=pst[:, 0:P])

                # causal attention, S^T layout (keys on partitions).
                # Diagonal tiles (kt = 4*qb+di) only contribute to query
                # columns >= 128*di of the block; narrow S/exp/O/R to the
                # live columns. Only the first 128 columns of a (narrowed)
                # diagonal tile are triangular; the rest are fully allowed.
                for qb in range(TB):
                    nkt = KT_PER_B * (qb + 1)
                    for kt in range(KT_PER_B * qb, nkt):
                        emit_transpose(kt)
                    psO_t = psO.tile([P, 512], F32, name="psodt", tag="pso")
                    racc = work.tile([P, 512], F32, name="racc", tag="racc")
                    for kt in range(nkt):
                        di = kt - KT_PER_B * qb
                        c0 = max(di, 0) * P  # first live query column
                        w = 512 - c0
                        qs = slice(qb * 512 + c0, (qb + 1) * 512)
                        psS_t = psS.tile([P, 512], F32, name="pssc", tag="pss")
                        nc.tensor.matmul(psS_t[:, :w],
                                         lhsT=kt_[:, kt * P:(kt + 1) * P],
                                         rhs=qt_[:, qs],
                                         start=True, stop=True)
                        pt = work.tile([P, 512], BF16, name="pexp", tag="pt",
                                       bufs=6)
                        nc.scalar.activation(
                            out=pt[:, :w], in_=psS_t[:, :w],
                            func=mybir.ActivationFunctionType.Exp, scale=s_exp)
                        if di >= 0:
                            nc.vector.tensor_mul(pt[:, :P], pt[:, :P],
                                                 cmask[:, 0, :P])
                        nc.tensor.matmul(psO_t[:, c0:], lhsT=vh[:, kt, :],
                                         rhs=pt[:, :w],
                                         start=(kt == 0), stop=(kt == nkt - 1),
                                         skip_group_check=True)
                        # fp32 running key-tile sum on DVE (hidden behind the
                        # exp pacing); one ones-matmul at the end reduces
                        # across partitions and broadcasts
                        if kt == 0:
                            nc.vector.tensor_copy(out=racc, in_=pt)
                        else:
                            nc.vector.tensor_add(racc[:, c0:], racc[:, c0:],
                                                 pt[:, :w])
                        pump_fills(1 + (kt & 1))
                    raccb = work.tile([P, 512], BF16, name="raccb", tag="raccb")
                    # 1/gamma_v folded here: rec = gamma_v / rowsum
                    nc.vector.tensor_scalar_mul(raccb, racc, inv_gv)
                    psR_t = psR.tile([P, 512], F32, name="psrow", tag="psr")
                    nc.tensor.matmul(psR_t, lhsT=ones, rhs=raccb,
                                     start=True, stop=True)
                    rec = work.tile([P, 512], F32, name="rec", tag="rec")
                    nc.vector.reciprocal_approx_fast(out=rec, in_=psR_t)
                    nc.vector.tensor_mul(ot[h][:, qb * 512:(qb + 1) * 512],
                                         psO_t, rec)
                    if h == H - 1:
                        # this token block's ot columns are now complete for
                        # every head: its o_proj chains become fill work
                        fills.append(oproj_nb_gen(qb))
                    pump_fills(4)
                if h < H - 1:
                    # finish next head's projections before its attention
                    while fills:
                        pump_fills(64)
                cur_tiles = next_tiles
            # drain remaining o_proj work
            while fills:
                pump_fills(64)

    nc.compile()
    return nc


def _ternary(w):
    """BitLinear ternary part and scale: w_eff = tern * gamma."""
    w = np.asarray(w, dtype=np.float32)
    gamma = max(np.float32(np.abs(w).mean(dtype=np.float32)), np.float32(1e-5))
    tern = np.clip(np.round(w / gamma), -1.0, 1.0).astype(np.float32)
    return tern, np.float32(gamma)


def _causal_masks():
    k = np.arange(128)[:, None]
    q = np.arange(512)[None, :]
    m = np.stack([(k <= q - 128 * i) for i in range(4)]).astype(np.float32)
    return m.astype(ml_dtypes.bfloat16)


def _tile_qkv(w_shard, k0, k1, dt):
    """[F, D] k-tiles k0..k1 -> [H, 128, n*128]:
    [h, p, j*128+f] = w_shard[h*128+f, (k0+j)*128+p]."""
    Fs = w_shard.shape[0]
    a = w_shard[:, k0 * 128:k1 * 128]
    a = a.reshape(Fs // 128, 128, k1 - k0, 128)  # [h, f, j, p]
    a = a.transpose(0, 3, 2, 1).reshape(Fs // 128, 128, (k1 - k0) * 128)
    return np.ascontiguousarray(a).astype(dt)


def _tile_wo(wo_shard):
    """[D, F] -> [MT, 128, H*128]: [m, p, h*128+j] = wo_shard[m*128+j, h*128+p]."""
    Ds, Fs = wo_shard.shape
    a = wo_shard.reshape(Ds // 128, 128, Fs // 128, 128)  # [m, j, h, p]
    a = a.transpose(0, 3, 2, 1).reshape(Ds // 128, 128, Fs)
    return np.ascontiguousarray(a)


def _prep_inputs(x, wq, wk, wv, wo):
    bf = ml_dtypes.bfloat16
    f8 = ml_dtypes.float8_e4m3
    x = np.asarray(x, dtype=np.float32)
    tq, gq = _ternary(wq)
    tk, gk = _ternary(wk)
    tv, gv = _ternary(wv)
    to, go = _ternary(wo)
    wo_eff = to * go
    cmask = _causal_masks()
    KD = D_MODEL // 128
    NB = KD - N1
    xTs = []
    for b in range(B):
        xT = np.ascontiguousarray(x[b].T)  # [D, T]
        x8 = xT[:N1 * 128].reshape(N1, 128, T_FULL).transpose(1, 0, 2)
        xbb = xT[N1 * 128:].reshape(NB, 128, T_FULL).transpose(1, 0, 2)
        xTs.append((np.ascontiguousarray(x8).astype(f8),
                    np.ascontiguousarray(xbb).astype(bf)))
    shards = []
    for hg in range(2):
        rows = slice(hg * F_LOC, (hg + 1) * F_LOC)
        H = F_LOC // 128
        # pack V,Q,K per head: [H, 128, 3, n*128] -> [H, 128, 3*n*128]
        w8 = np.stack([_tile_qkv(t[rows, :], 0, N1, f8)
                       for t in (tv, tq, tk)], axis=2)
        wb = np.stack([_tile_qkv(t[rows, :], N1, KD, bf)
                       for t in (tv, tq, tk)], axis=2)
        shards.append({
            "w8": np.ascontiguousarray(w8).reshape(H, 128, 3 * N1 * 128),
            "wb": np.ascontiguousarray(wb).reshape(H, 128, 3 * NB * 128),
            "woT": _tile_wo(wo_eff[:, rows]).astype(bf),
        })
    in_maps = []
    for c in range(N_CORES):
        b, hg = c // 2, c % 2
        m = {"x8": xTs[b][0], "xb": xTs[b][1], "cmask": cmask}
        m.update(shards[hg])
        in_maps.append(m)
    scale = np.float32(1.0 / math.sqrt(D_HEAD))
    s_exp = np.float32(scale * gq * gk)
    inv_gv = np.float32(1.0 / gv)
    return in_maps, float(s_exp), float(inv_gv)


_NC_CACHE = {}


def _get_nc(s_exp, inv_gv):
    key = (s_exp, inv_gv)
    if key not in _NC_CACHE:
        _NC_CACHE[key] = build_bass(s_exp, inv_gv)
    return _NC_CACHE[key]


def run(x, wq, wk, wv, wo, trace=False):
    in_maps, s_exp, inv_gv = _prep_inputs(x, wq, wk, wv, wo)
    nc = _get_nc(s_exp, inv_gv)
    res = bass_utils.run_bass_kernel_spmd(
        nc, in_maps, core_ids=list(range(N_CORES)), trace=trace)
    out = np.empty((B, T_FULL, D_MODEL), dtype=np.float32)
    for b in range(B):
        out[b] = (res.results[2 * b]["outT"].astype(np.float32)
                  + res.results[2 * b + 1]["outT"].astype(np.float32)).T
    return out, res


def kernel(x, wq, wk, wv, wo):
    out, _ = run(x, wq, wk, wv, wo)
    return out


# revision 25
# speedup vs baseline: 1.1771x; 1.1771x over previous
"""BitSelfAttention on 8 TRN2 NeuronCores.

Sharding: core c handles batch b = c//2 and head-group hg = c%2 (8 of 16 heads).
Each core computes its 8 heads' QKV projections + causal attention + its slice
of the o_proj GEMM, producing a partial output (transposed, [D, T], fp32).
Host sums the two head-group partials per batch at the end.

Speed: the QKV projection GEMMs run a mixed-precision contraction: the first
N1=6 of 16 k-tiles of x are fp8(e4m3) and processed pairwise with
perf_mode=DoubleRow (2 fp8 MACs/cell/cycle, ~1.7x bf16 for those tiles); the
remaining 10 k-tiles stay bf16. BitLinear weights are pure ternary {-1,0,+1}
— exact in both fp8 and bf16 — so the only quantization error is on the fp8
slice of x (measured end-to-end rel-err ~1.5e-2 vs the 2e-2 gate).
gamma scales are applied exactly in fp32: gamma_q*gamma_k folds into the
softmax exp scale, gamma_v into the row-sum normalization, gamma_o stays
folded into the bf16 o_proj weights.

Device layouts (per core):
  x8   [128, N1, T] fp8 : x[b].T k-tiles 0..N1 (DoubleRow pairs on dim 1)
  xT   [NB*128, T] bf16 : x[b].T k-tiles N1..16
  w*8  [H, 128, N1*128] fp8  : ternary weight k-tiles 0..N1 (per head)
  w*T  [H, 128, NB*128] bf16 : ternary weight k-tiles N1..16
  woT  [MT, 128, H*128] bf16 : w_o_eff[:, hg-cols].T (gamma_o folded)
  cmask[4, 128, 512] bf16 : causal masks for the 4 diagonal offsets
  outT [D, T]  fp32 : partial output, transposed

Per head h: Q^T,K^T [dh=128, T] (dh-major), V^T transposed on the PE into
token-major V tiles. Attention computed as S^T = K^T_tile.T @ Q^T_block so
softmax rows land on the free axis; P^T = exp(S^T*scale) (ACT, PSUM->SBUF
bf16); key-tile partial row-sums accumulate in fp32 on the vector engine and
one all-ones stationary matmul per block reduces across partitions while
broadcasting the result to every partition; O^T = V_tile.T @ P^T accumulated
over key tiles; normalize with fast-reciprocal+multiply during PSUM eviction.
o_proj consumes O^T tiles directly as stationary operands, producing outT; its
per-token-block chains double as PE fill work zipped into the last head's
attention, just as each head's projection chains are zipped into the previous
head's attention (the attention inner loop is otherwise exp-latency-gated on
the in-order PE).
"""

import math

import ml_dtypes
import numpy as np

import concourse.mybir as mybir
import concourse.tile as tile
from concourse import bacc
from concourse import bass_utils
from concourse.masks import make_identity

BF16 = mybir.dt.bfloat16
FP8 = mybir.dt.float8e4
F32 = mybir.dt.float32

D_MODEL = 2048
N_HEAD = 16
D_HEAD = 128
B = 4
T_FULL = 2048
N_CORES = 8
F_LOC = D_MODEL // 2  # features per core (8 heads)
N1 = 6                # fp8 k-tiles (of 16) in the QKV projection contraction


def build_bass(s_exp, inv_gv, T=T_FULL, D=D_MODEL, F=F_LOC, debug=False):
    """Build the single-core program (SPMD across 8 cores via input data).

    s_exp: softmax exp scale = gamma_q*gamma_k/sqrt(dh) (fp32 immediate)
    inv_gv: 1/gamma_v, folded into the row-sum (fp32 immediate)
    """
    P = 128
    KD = D // P      # contraction 128-tiles
    NB = KD - N1     # bf16 k-tiles
    TT = T // P      # token 128-tiles
    TB = T // 512    # token 512-blocks
    H = F // P       # local heads
    MT = D // P      # output-dmodel 128-tiles
    KT_PER_B = 512 // P
    DR = mybir.MatmulPerfMode.DoubleRow

    nc = bacc.Bacc("TRN2", target_bir_lowering=False, debug=debug,
                   num_devices=N_CORES)
    x8_d = nc.dram_tensor("x8", [P, N1, T], FP8, kind="ExternalInput").ap()
    xT_d = nc.dram_tensor("xT", [NB * P, T], BF16, kind="ExternalInput").ap()
    # weights pre-tiled on host into the exact SBUF layouts (contiguous DMAs,
    # one tensor per projection part so the loads spread across DMA queues):
    #   w*8: [H, 128, N1*128] with [h, p, j*128+f] = w_tern[h*128+f, j*128+p]
    #   w*T: [H, 128, NB*128] with [h, p, kd*128+f] = w_tern[h*128+f, (N1+kd)*128+p]
    #   woT: [MT, 128, H*128] with [m, p, h*128+j] = wo_eff[m*128+j, h*128+p]
    wq8_d = nc.dram_tensor("wq8", [H, P, N1 * P], FP8,
                           kind="ExternalInput").ap()
    wk8_d = nc.dram_tensor("wk8", [H, P, N1 * P], FP8,
                           kind="ExternalInput").ap()
    wv8_d = nc.dram_tensor("wv8", [H, P, N1 * P], FP8,
                           kind="ExternalInput").ap()
    wqT_d = nc.dram_tensor("wqT", [H, P, NB * P], BF16,
                           kind="ExternalInput").ap()
    wkT_d = nc.dram_tensor("wkT", [H, P, NB * P], BF16,
                           kind="ExternalInput").ap()
    wvT_d = nc.dram_tensor("wvT", [H, P, NB * P], BF16,
                           kind="ExternalInput").ap()
    woT_d = nc.dram_tensor("woT", [MT, P, H * P], BF16,
                           kind="ExternalInput").ap()
    cm_d = nc.dram_tensor("cmask", [4, P, 512], BF16, kind="ExternalInput").ap()
    # bf16 partials: host sums the two head-group halves in fp32; the bf16
    # rounding of each partial is ~2e-4 of the output absmax — negligible —
    # and it halves the 16MB output DMA stream.
    out_d = nc.dram_tensor("outT", [D, T], BF16, kind="ExternalOutput").ap()

    with tile.TileContext(nc) as tc:
        with (
            tc.tile_pool(name="big", bufs=1) as big,
            tc.tile_pool(name="work", bufs=2) as work,
            tc.tile_pool(name="psS", bufs=3, space="PSUM") as psS,
            tc.tile_pool(name="psO", bufs=2, space="PSUM") as psO,
            tc.tile_pool(name="psR", bufs=1, space="PSUM") as psR,
            tc.tile_pool(name="psP", bufs=2, space="PSUM") as psP,
        ):
            # ---- head-0 weights first: the first matmul chains need them.
            # bufs=4 on the weight tags: heads h..h+3 alive (prefetch 3 deep)
            def load_head_weights(h, wv=None):
                if wv is None:
                    wv8 = work.tile([P, N1, P], FP8, name=f"wv8{h}",
                                    tag="wv8", bufs=4)
                    nc.sync.dma_start(out=wv8.rearrange("p n f -> p (n f)"),
                                      in_=wv8_d[h])
                    wvh = work.tile([P, NB, P], BF16, name=f"wvh{h}",
                                    tag="wvh", bufs=4)
                    nc.sync.dma_start(out=wvh.rearrange("p kd f -> p (kd f)"),
                                      in_=wvT_d[h])
                    wv = (wv8, wvh)
                wq8 = work.tile([P, N1, P], FP8, name=f"wq8{h}", tag="wq8",
                                bufs=4)
                nc.sync.dma_start(out=wq8.rearrange("p n f -> p (n f)"),
                                  in_=wq8_d[h])
                wqh = work.tile([P, NB, P], BF16, name=f"wqh{h}", tag="wqh",
                                bufs=4)
                nc.sync.dma_start(out=wqh.rearrange("p kd f -> p (kd f)"),
                                  in_=wqT_d[h])
                wk8 = work.tile([P, N1, P], FP8, name=f"wk8{h}", tag="wk8",
                                bufs=4)
                nc.sync.dma_start(out=wk8.rearrange("p n f -> p (n f)"),
                                  in_=wk8_d[h])
                wkh = work.tile([P, NB, P], BF16, name=f"wkh{h}", tag="wkh",
                                bufs=4)
                nc.sync.dma_start(out=wkh.rearrange("p kd f -> p (kd f)"),
                                  in_=wkT_d[h])
                return ((wq8, wqh), (wk8, wkh), wv)

            wv8h0 = work.tile([P, N1, P], FP8, name="wv8h0", tag="wv8",
                              bufs=4)
            nc.sync.dma_start(out=wv8h0.rearrange("p n f -> p (n f)"),
                              in_=wv8_d[0])
            wvh0 = work.tile([P, NB, P], BF16, name="wvh0", tag="wvh",
                             bufs=4)
            nc.sync.dma_start(out=wvh0.rearrange("p kd f -> p (kd f)"),
                              in_=wvT_d[0])
            ones = big.tile([P, P], BF16, name="ones_sb", tag="ones", bufs=1)
            nc.vector.memset(ones, 1.0)
            ident = big.tile([P, P], BF16, name="ident_sb", tag="ident", bufs=1)
            make_identity(nc, ident)
            # ---- x: chunked by 512-token block so the first projection
            # chains start after ~1/4 of the x DMA instead of all of it.
            # Block 0 is issued first; head-0/1 weights go next (ahead of the
            # remaining x blocks) so the early chains never wait on a
            # weight DMA queued behind 5MB of x. The bf16 part stays as 10
            # separate [128, T] tiles so its chunks spread across DMA queues.
            x8 = big.tile([P, N1, T], FP8, name="x8_sb", tag="x8", bufs=1)
            xt = [big.tile([P, T], BF16, name=f"xt{kd}", tag="xt", bufs=NB)
                  for kd in range(NB)]

            def dma_x_block(tb):
                ts_ = slice(tb * 512, (tb + 1) * 512)
                nc.sync.dma_start(out=x8[:, :, ts_], in_=x8_d[:, :, ts_])
                for kd in range(NB):
                    nc.sync.dma_start(out=xt[kd][:, ts_],
                                      in_=xT_d[kd * P:(kd + 1) * P, ts_])

            dma_x_block(0)
            cmask = big.tile([P, 4, 512], BF16, name="cmask_sb", tag="cmask",
                             bufs=1)
            for i in range(4):
                nc.sync.dma_start(out=cmask[:, i, :], in_=cm_d[i])
            ot = [big.tile([P, T], BF16, name=f"ot{h}", tag="ot", bufs=H)
                  for h in range(H)]

            # ---- per-head pipeline with cross-head fill interleaving.
            # The attention inner loop is ACT(exp)-gated by ~40ns/iter; we
            # pump one projection matmul of the NEXT head between attention
            # iterations so the (in-order) PE always has fill work.
            def alloc_head_tiles(h):
                vT = work.tile([P, T], BF16, name=f"vT{h}", tag="vT")
                vh = work.tile([P, TT, P], BF16, name=f"vh{h}", tag="vh")
                qt_ = work.tile([P, T], BF16, name=f"qt{h}", tag="qt")
                kt_ = work.tile([P, T], BF16, name=f"kt{h}", tag="kt")
                return vT, vh, qt_, kt_

            def proj_fill_gen(ws, tiles):
                """V^T/Q^T/K^T projection chains in token-block-outer order
                (so head 0 tracks the chunked x DMA arrival), yielding after
                every matmul so the caller can interleave them. Each chain:
                NB bf16 matmuls + N1/2 fp8 DoubleRow matmuls into one PSUM
                accumulation group."""
                wsq, wsk, wsv = ws
                vT, vh, qt_, kt_ = tiles
                for tb in range(TB):
                    for (w8, wb), dst in ((wsv, vT), (wsq, qt_), (wsk, kt_)):
                        ts_ = slice(tb * 512, (tb + 1) * 512)
                        ps = psP.tile([P, 512], F32, name="psfill", tag="psp")
                        for kd in range(NB):
                            nc.tensor.matmul(ps, lhsT=wb[:, kd, :],
                                             rhs=xt[kd][:, ts_],
                                             start=(kd == 0), stop=False)
                            yield
                        for j in range(N1 // 2):
                            nc.tensor.matmul(
                                ps, lhsT=w8[:, 2 * j:2 * j + 2, :],
                                rhs=x8[:, 2 * j:2 * j + 2, ts_],
                                start=False, stop=(j == N1 // 2 - 1),
                                perf_mode=DR)
                            yield
                        nc.vector.tensor_copy(out=dst[:, ts_], in_=ps)

            def pump(gen, n):
                for _ in range(n):
                    try:
                        next(gen)
                    except StopIteration:
                        return False
                return True

            def pump_n(gen, n):
                c = 0
                for _ in range(n):
                    try:
                        next(gen)
                        c += 1
                    except StopIteration:
                        break
                return c

            def oproj_nb_gen(nb):
                """o_proj chains for one token block (needs all heads' ot
                columns of that block only), yielding per matmul. woh weight
                tiles are prefetched two m-tiles ahead so the chains never
                stall the PE on their DMA."""
                ns = slice(nb * 512, (nb + 1) * 512)
                wohs = {}

                def load(m):
                    woh = work.tile([P, H, P], BF16, name=f"woh{nb}_{m}",
                                    tag="woh", bufs=4)
                    nc.sync.dma_start(out=woh.rearrange("p h f -> p (h f)"),
                                      in_=woT_d[m])
                    return woh

                wohs[0] = load(0)
                wohs[1] = load(1)
                yield  # let attention matmuls cover the woh DMA latency
                for m in range(MT):
                    if m + 2 < MT:
                        wohs[m + 2] = load(m + 2)
                    ps = psP.tile([P, 512], F32, name="psout", tag="psp")
                    for hh in range(H):
                        nc.tensor.matmul(ps, lhsT=wohs[m][:, hh, :],
                                         rhs=ot[hh][:, ns],
                                         start=(hh == 0), stop=(hh == H - 1))
                        yield
                    del wohs[m]
                    stg = work.tile([P, 512], BF16, name="ostage",
                                    tag="ostage", bufs=4)
                    nc.vector.tensor_copy(out=stg, in_=ps)
                    nc.sync.dma_start(out=out_d[m * P:(m + 1) * P, ns],
                                      in_=stg)

            # head-0 Q/K and head-1 weights load right after x block 0; the
            # remaining x blocks stream behind them
            ws_list = [None] * (H + 3)
            ws_list[0] = load_head_weights(0, wv=(wv8h0, wvh0))
            if H > 1:
                ws_list[1] = load_head_weights(1)
            for tb in range(1, TB):
                dma_x_block(tb)
            cur_tiles = alloc_head_tiles(0)
            g0 = proj_fill_gen(ws_list[0], cur_tiles)
            while pump(g0, 1):
                pass

            fills = []

            def pump_fills(n):
                while n > 0 and fills:
                    n -= pump_n(fills[0], n)
                    if n > 0:
                        fills.pop(0)

            for h in range(H):
                vT, vh, qt_, kt_ = cur_tiles
                # prefetch weights three heads ahead so fill matmuls never
                # wait on their DMA (a blocked fill stalls the in-order PE)
                if h == 0 and H > 2:
                    ws_list[2] = load_head_weights(2)
                if h + 3 < H:
                    ws_list[h + 3] = load_head_weights(h + 3)
                if h + 1 < H:
                    next_tiles = alloc_head_tiles(h + 1)
                    fills.append(proj_fill_gen(ws_list[h + 1], next_tiles))
                else:
                    next_tiles = None

                def emit_transpose(kt):
                    # lives in the psS pool: psP slots are held long by
                    # in-flight interleaved fill chains
                    pst = psS.tile([P, 512], BF16, name="pst", tag="pss")
                    nc.tensor.transpose(pst[:, 0:P],
                                        vT[:, kt * P:(kt + 1) * P], ident)
                    nc.vector.tensor_copy(out=vh[:, kt, :], in_=pst[:, 0:P])

                # causal attention, S^T layout (keys on partitions).
                # Diagonal tiles (kt = 4*qb+di) only contribute to query
                # columns >= 128*di of the block; narrow S/exp/O/R to the
                # live columns. Only the first 128 columns of a (narrowed)
                # diagonal tile are triangular; the rest are fully allowed.
                for qb in range(TB):
                    nkt = KT_PER_B * (qb + 1)
                    for kt in range(KT_PER_B * qb, nkt):
                        emit_transpose(kt)
                    psO_t = psO.tile([P, 512], F32, name="psodt", tag="pso")
                    racc = work.tile([P, 512], F32, name="racc", tag="racc")
                    for kt in range(nkt):
                        di = kt - KT_PER_B * qb
                        c0 = max(di, 0) * P  # first live query column
                        w = 512 - c0
                        qs = slice(qb * 512 + c0, (qb + 1) * 512)
                        psS_t = psS.tile([P, 512], F32, name="pssc", tag="pss")
                        nc.tensor.matmul(psS_t[:, :w],
                                         lhsT=kt_[:, kt * P:(kt + 1) * P],
                                         rhs=qt_[:, qs],
                                         start=True, stop=True)
                        pt = work.tile([P, 512], BF16, name="pexp", tag="pt",
                                       bufs=6)
                        nc.scalar.activation(
                            out=pt[:, :w], in_=psS_t[:, :w],
                            func=mybir.ActivationFunctionType.Exp, scale=s_exp)
                        if di >= 0:
                            nc.vector.tensor_mul(pt[:, :P], pt[:, :P],
                                                 cmask[:, 0, :P])
                        nc.tensor.matmul(psO_t[:, c0:], lhsT=vh[:, kt, :],
                                         rhs=pt[:, :w],
                                         start=(kt == 0), stop=(kt == nkt - 1),
                                         skip_group_check=True)
                        # fp32 running key-tile sum on DVE (hidden behind the
                        # exp pacing); one ones-matmul at the end reduces
                        # across partitions and broadcasts
                        if kt == 0:
                            nc.vector.tensor_copy(out=racc, in_=pt)
                        else:
                            nc.vector.tensor_add(racc[:, c0:], racc[:, c0:],
                                                 pt[:, :w])
                        pump_fills(1 + (kt & 1))
                    raccb = work.tile([P, 512], BF16, name="raccb", tag="raccb")
                    # 1/gamma_v folded here: rec = gamma_v / rowsum
                    nc.vector.tensor_scalar_mul(raccb, racc, inv_gv)
                    psR_t = psR.tile([P, 512], F32, name="psrow", tag="psr")
                    nc.tensor.matmul(psR_t, lhsT=ones, rhs=raccb,
                                     start=True, stop=True)
                    rec = work.tile([P, 512], F32, name="rec", tag="rec")
                    nc.vector.reciprocal_approx_fast(out=rec, in_=psR_t)
                    nc.vector.tensor_mul(ot[h][:, qb * 512:(qb + 1) * 512],
                                         psO_t, rec)
                    if h == H - 1:
                        # this token block's ot columns are now complete for
                        # every head: its o_proj chains become fill work
                        fills.append(oproj_nb_gen(qb))
                    pump_fills(4)
                if h < H - 1:
                    # finish next head's projections before its attention
                    while fills:
                        pump_fills(64)
                cur_tiles = next_tiles
            # drain remaining o_proj work
            while fills:
                pump_fills(64)

    nc.compile()
    return nc


def _ternary(w):
    """BitLinear ternary part and scale: w_eff = tern * gamma."""
    w = np.asarray(w, dtype=np.float32)
    gamma = max(np.float32(np.abs(w).mean(dtype=np.float32)), np.float32(1e-5))
    tern = np.clip(np.round(w / gamma), -1.0, 1.0).astype(np.float32)
    return tern, np.float32(gamma)


def _causal_masks():
    k = np.arange(128)[:, None]
    q = np.arange(512)[None, :]
    m = np.stack([(k <= q - 128 * i) for i in range(4)]).astype(np.float32)
    return m.astype(ml_dtypes.bfloat16)


def _tile_qkv(w_shard, k0, k1, dt):
    """[F, D] k-tiles k0..k1 -> [H, 128, n*128]:
    [h, p, j*128+f] = w_shard[h*128+f, (k0+j)*128+p]."""
    Fs = w_shard.shape[0]
    a = w_shard[:, k0 * 128:k1 * 128]
    a = a.reshape(Fs // 128, 128, k1 - k0, 128)  # [h, f, j, p]
    a = a.transpose(0, 3, 2, 1).reshape(Fs // 128, 128, (k1 - k0) * 128)
    return np.ascontiguousarray(a).astype(dt)


def _tile_wo(wo_shard):
    """[D, F] -> [MT, 128, H*128]: [m, p, h*128+j] = wo_shard[m*128+j, h*128+p]."""
    Ds, Fs = wo_shard.shape
    a = wo_shard.reshape(Ds // 128, 128, Fs // 128, 128)  # [m, j, h, p]
    a = a.transpose(0, 3, 2, 1).reshape(Ds // 128, 128, Fs)
    return np.ascontiguousarray(a)


def _prep_inputs(x, wq, wk, wv, wo):
    bf = ml_dtypes.bfloat16
    f8 = ml_dtypes.float8_e4m3
    x = np.asarray(x, dtype=np.float32)
    tq, gq = _ternary(wq)
    tk, gk = _ternary(wk)
    tv, gv = _ternary(wv)
    to, go = _ternary(wo)
    wo_eff = to * go
    cmask = _causal_masks()
    KD = D_MODEL // 128
    xTs = []
    for b in range(B):
        xT = np.ascontiguousarray(x[b].T)  # [D, T]
        x8 = xT[:N1 * 128].reshape(N1, 128, T_FULL).transpose(1, 0, 2)
        xTs.append((np.ascontiguousarray(x8).astype(f8),
                    xT[N1 * 128:].astype(bf)))
    shards = []
    for hg in range(2):
        rows = slice(hg * F_LOC, (hg + 1) * F_LOC)
        shards.append({
            "wq8": _tile_qkv(tq[rows, :], 0, N1, f8),
            "wk8": _tile_qkv(tk[rows, :], 0, N1, f8),
            "wv8": _tile_qkv(tv[rows, :], 0, N1, f8),
            "wqT": _tile_qkv(tq[rows, :], N1, KD, bf),
            "wkT": _tile_qkv(tk[rows, :], N1, KD, bf),
            "wvT": _tile_qkv(tv[rows, :], N1, KD, bf),
            "woT": _tile_wo(wo_eff[:, rows]).astype(bf),
        })
    in_maps = []
    for c in range(N_CORES):
        b, hg = c // 2, c % 2
        m = {"x8": xTs[b][0], "xT": xTs[b][1], "cmask": cmask}
        m.update(shards[hg])
        in_maps.append(m)
    scale = np.float32(1.0 / math.sqrt(D_HEAD))
    s_exp = np.float32(scale * gq * gk)
    inv_gv = np.float32(1.0 / gv)
    return in_maps, float(s_exp), float(inv_gv)


_NC_CACHE = {}


def _get_nc(s_exp, inv_gv):
    key = (s_exp, inv_gv)
    if key not in _NC_CACHE:
        _NC_CACHE[key] = build_bass(s_exp, inv_gv)
    return _NC_CACHE[key]


def run(x, wq, wk, wv, wo, trace=False):
    in_maps, s_exp, inv_gv = _prep_inputs(x, wq, wk, wv, wo)
    nc = _get_nc(s_exp, inv_gv)
    res = bass_utils.run_bass_kernel_spmd(
        nc, in_maps, core_ids=list(range(N_CORES)), trace=trace)
    out = np.empty((B, T_FULL, D_MODEL), dtype=np.float32)
    for b in range(B):
        out[b] = (res.results[2 * b]["outT"].astype(np.float32)
                  + res.results[2 * b + 1]["outT"].astype(np.float32)).T
    return out, res


def kernel(x, wq, wk, wv, wo):
    out, _ = run(x, wq, wk, wv, wo)
    return out
